# revision 1
# baseline (speedup 1.0000x reference)
"""DNA Transport Hamiltonian GNN kernel for Trainium2 (8 NeuronCores).

Builds [8, 2048, 2048] banded Hamiltonians. Sharding: one graph per core;
MLP weights replicated. The output is 99.6% zeros (9 diagonals only), so the
kernel streams the zero background out of a constant SBUF tile (no compute
dependency) while the PE computes the two small MLPs and assembles the
9-diagonal band windows.

Hardcoded problem structure (from the generating module):
  B=8 graphs, 2048 DNA nodes/graph (+2 contact nodes at graph start),
  HID=128, edges per graph: (i, i+d) for d=1..4 -> 2047+2046+2045+2044=8182,
  laid out d-major per graph, graphs contiguous.
"""

import numpy as np

B = 8
ND = 2048            # DNA nodes per graph == H_size
NPG = ND + 2         # nodes per graph incl. 2 contacts
HID = 128
EP = 8182            # edges per graph
EPAD = 8192
NT = ND // 128       # 16 row blocks
OFF = {1: 0, 2: 2047, 3: 4093, 4: 6138}   # start of band d in per-graph edge order
WIN = 136            # band window width: 128 + 2*4

_PROG = None


def _build_program():
    import concourse.bass as bass
    import concourse.tile as tile
    from concourse.tile import add_dep_helper
    from concourse import mybir
    from contextlib import ExitStack

    f32 = mybir.dt.float32
    f32r = mybir.dt.float32r
    Alu = mybir.AluOpType
    Act = mybir.ActivationFunctionType

    nc = bass.Bass()

    xt = nc.declare_dram_parameter("xt", [HID, ND], f32, isOutput=False)
    eft = nc.declare_dram_parameter("eft", [HID, EPAD], f32, isOutput=False)
    wo1 = nc.declare_dram_parameter("wo1", [HID, HID], f32, isOutput=False)
    wc1 = nc.declare_dram_parameter("wc1", [HID, HID], f32, isOutput=False)
    bo1 = nc.declare_dram_parameter("bo1", [HID, 1], f32, isOutput=False)
    bc1 = nc.declare_dram_parameter("bc1", [HID, 1], f32, isOutput=False)
    w2 = nc.declare_dram_parameter("w2", [HID, 2], f32, isOutput=False)  # col0=wc2, col1=wo2
    mask0 = nc.declare_dram_parameter("mask0", [128, 144], f32, isOutput=False)
    biasc = nc.declare_dram_parameter("biasc", [128, 9 * NT], f32, isOutput=False)
    zv = nc.declare_dram_parameter("zv", [128, 1916], f32, isOutput=False)
    h = nc.declare_dram_parameter("h", [ND, ND], f32, isOutput=True)

    with tile.TileContext(nc) as tc, ExitStack() as ctx:
        cons = ctx.enter_context(tc.tile_pool(name="cons", bufs=1))
        psL1 = ctx.enter_context(tc.tile_pool(name="psL1", bufs=2, space="PSUM"))
        psRow = ctx.enter_context(tc.tile_pool(name="psRow", bufs=2, space="PSUM"))
        psPers = ctx.enter_context(tc.tile_pool(name="psPers", bufs=1, space="PSUM"))
        # bufs = NT so slots are never reused: avoids WAR release semaphores
        # that would push PE/fp32-matmul instructions past their 1-wait limit
        cpool = ctx.enter_context(tc.tile_pool(name="cpool", bufs=NT))

        # ---- constant / persistent tiles ----
        XT = cons.tile([HID, ND], f32)
        EFT = cons.tile([HID, EPAD], f32)
        WO1 = cons.tile([HID, HID], f32)
        WC1 = cons.tile([HID, HID], f32)
        BO1 = cons.tile([HID, 1], f32)
        BC1 = cons.tile([HID, 1], f32)
        W2 = cons.tile([HID, 2], f32)
        MASK = cons.tile([128, 144], f32)
        BIASC = cons.tile([128, 9 * NT], f32)
        H1ET = cons.tile([HID, EPAD], f32)
        H1XT = cons.tile([HID, ND], f32)
        RE = cons.tile([1, 4 + EPAD], f32)   # coupling row, 4 leading zeros
        RX = cons.tile([1, ND], f32)         # onsite row
        ONE1 = cons.tile([1, 1], f32)
        SCRA = cons.tile([1, 2], f32)        # ACT warmup scratch
        SCRD = cons.tile([1, 2], f32)        # DVE warmup scratch
        Z = cons.tile([128, 1916], f32)      # zero background source

        # Z first: it lands on HWDGE queue 0, and the first zero DMA is the
        # 9th HWDGE DMA -> also queue 0, so its Z dependency and queue-FIFO
        # wait are the same semaphore (DMA instructions take 1 sync wait).
        nc.sync.dma_start(Z[:], zv[:])
        nc.sync.dma_start(WO1[:], wo1[:])
        nc.sync.dma_start(WC1[:], wc1[:])
        nc.sync.dma_start(BO1[:], bo1[:])
        nc.sync.dma_start(BC1[:], bc1[:])
        nc.sync.dma_start(W2[:], w2[:])
        nc.sync.dma_start(MASK[:], mask0[:])
        nc.sync.dma_start(BIASC[:], biasc[:])
        # 9th HWDGE DMA -> queue 0 (same as Z): first zero-background DMA
        zero_dmas = [nc.sync.dma_start(h[0:128, 132:ND], Z[:, 0:1916])]
        # feature loads chunked across queues so layer-1 can start on chunk 0
        # ~1.5us in instead of after one serial 4MB transfer
        for j in range(EPAD // 512):
            nc.sync.dma_start(EFT[:, 512 * j:512 * (j + 1)],
                              eft[:, 512 * j:512 * (j + 1)])
        for j in range(ND // 512):
            nc.sync.dma_start(XT[:, 512 * j:512 * (j + 1)],
                              xt[:, 512 * j:512 * (j + 1)])

        # ---- bulk zero background: no compute dependency, overlaps everything
        for t in range(NT):
            r0 = 128 * t
            lw = r0 - 4
            if t >= 1:
                zero_dmas.append(
                    nc.sync.dma_start(h[r0:r0 + 128, 0:lw], Z[:, 0:lw]))
            rw = 1916 - r0
            if 1 <= t <= NT - 2:
                zero_dmas.append(
                    nc.sync.dma_start(h[r0:r0 + 128, r0 + 132:ND], Z[:, 0:rw]))

        # ---- semaphore warmups: each engine observes every input-DMA queue
        # via ops with exactly one wait, so no later instruction (esp. fp32
        # matmuls, which take a single sync wait) needs >1 wait.
        pd = psPers.tile([1, 28], f32)
        nc.tensor.matmul(pd[0:1, 0:1], WC1[0:1, 0:1], WC1[0:1, 0:1],
                         start=True, stop=True)
        nc.tensor.matmul(pd[0:1, 1:2], WO1[0:1, 0:1], WO1[0:1, 0:1],
                         start=True, stop=True)
        nc.tensor.matmul(pd[0:1, 2:3], W2[0:1, 0:1], W2[0:1, 0:1],
                         start=True, stop=True)
        # ACT: absorb BC1/BO1/XT queues; produce ONE1 and RE's 4-col zero pad
        nc.scalar.activation(ONE1[0:1, 0:1], BC1[0:1, 0:1], Act.Copy,
                             bias=1.0, scale=0.0)
        nc.scalar.activation(SCRA[0:1, 0:1], BO1[0:1, 0:1], Act.Copy,
                             bias=0.0, scale=0.0)
        nc.scalar.activation(RE[0:1, 0:4], XT[0:1, 0:4], Act.Copy,
                             bias=0.0, scale=0.0)
        # DVE: absorb MASK/BIASC queues
        nc.vector.tensor_copy(SCRD[0:1, 0:1], MASK[0:1, 0:1])
        nc.vector.tensor_copy(SCRD[0:1, 1:2], BIASC[0:1, 0:1])

        # ---- pipelined compute: for each chunk group g, run layer-1 +
        # layer-2 on the four EFT chunks blocks 4g..4g+3 need (one per band
        # region) plus the XT chunk, then assemble+emit those blocks while
        # later groups are still computing. Per-chunk PE warmup matmuls
        # absorb each chunk-DMA queue semaphore (fp32 matmul 1-wait limit).
        PSA = psPers.tile([128, 76], f32)   # 72 band cols + spare col 72
        PSB = psPers.tile([128, 76], f32)
        # Windows merge into <=8 SWDGE DMAs so each lands on a fresh SWDGE
        # queue: exactly one sync wait (the DVE band-op semaphore).
        GROUPS = [(0, 1), (1, 4), (5, 4), (9, 4), (13, 2), (15, 1)]
        wt = {}
        for t0, nb in GROUPS:
            tile_w = cons.tile([128, nb * WIN], f32, tag=f"wg{t0}")
            for i in range(nb):
                wt[t0 + i] = (tile_w, i * WIN, t0, nb)
        window_dmas = []
        wcol = [3]
        lastd = {}

        def l1l2_edges(j):
            nc.tensor.matmul(pd[0:1, wcol[0]:wcol[0] + 1],
                             EFT[0:1, 512 * j:512 * j + 1],
                             EFT[0:1, 512 * j:512 * j + 1],
                             start=True, stop=True)
            wcol[0] += 1
            ps = psL1.tile([128, 512], f32)
            nc.tensor.matmul(ps[:], WC1[:], EFT[:, 512 * j:512 * (j + 1)],
                             start=True, stop=True)
            nc.scalar.activation(H1ET[:, 512 * j:512 * (j + 1)], ps[:],
                                 Act.Relu, bias=BC1[:, 0:1])
            ps2 = psRow.tile([1, 512], f32)
            nc.tensor.matmul(ps2[:], W2[:, 0:1],
                             H1ET[:, 512 * j:512 * (j + 1)],
                             start=True, stop=True)
            nc.scalar.copy(RE[0:1, 4 + 512 * j:4 + 512 * (j + 1)], ps2[:])

        def l1l2_nodes(g):
            nc.tensor.matmul(pd[0:1, wcol[0]:wcol[0] + 1],
                             XT[0:1, 512 * g:512 * g + 1],
                             XT[0:1, 512 * g:512 * g + 1],
                             start=True, stop=True)
            wcol[0] += 1
            ps = psL1.tile([128, 512], f32)
            nc.tensor.matmul(ps[:], WO1[:], XT[:, 512 * g:512 * (g + 1)],
                             start=True, stop=True)
            nc.scalar.activation(H1XT[:, 512 * g:512 * (g + 1)], ps[:],
                                 Act.Relu, bias=BO1[:, 0:1])
            ps2 = psRow.tile([1, 512], f32)
            nc.tensor.matmul(ps2[:], W2[:, 1:2],
                             H1XT[:, 512 * g:512 * (g + 1)],
                             start=True, stop=True)
            return nc.scalar.copy(RX[0:1, 512 * g:512 * (g + 1)], ps2[:])

        def emit_block(t):
            r0 = 128 * t
            ps = (PSA, PSB)[t % 2]
            c0 = 9 * (t // 2)
            # dummy write to the spare column: absorbs the PSUM-bank release
            # (DVE) semaphore so the real transposes only wait on ACT
            nc.tensor.transpose(ps[0:1, 72:73], ONE1[0:1, 0:1], ONE1[:])
            nc.tensor.transpose(ps[:, c0 + 4:c0 + 5], RX[0:1, r0:r0 + 128], ONE1[:])
            for d in range(1, 5):
                s = 4 + OFF[d] + r0
                nc.tensor.transpose(ps[:, c0 + 4 + d:c0 + 5 + d],
                                    RE[0:1, s:s + 128], ONE1[:])
                lastd['pe'] = nc.tensor.transpose(
                    ps[:, c0 + 4 - d:c0 + 5 - d],
                    RE[0:1, s - d:s - d + 128], ONE1[:])
            c = cpool.tile([128, 9], f32)
            nc.vector.tensor_tensor(c[:], ps[:, c0:c0 + 9],
                                    BIASC[:, 9 * t:9 * t + 9], op=Alu.add)
            tile_w, j0, t0, nb = wt[t]
            wsl = tile_w[:, j0:j0 + WIN]
            nc.vector.tensor_scalar_mul(wsl, MASK[:, 8:8 + WIN], c[:, 0:1])
            for g in range(1, 9):
                lb = nc.vector.scalar_tensor_tensor(
                    wsl, MASK[:, 8 - g:8 - g + WIN], c[:, g:g + 1], wsl,
                    op0=Alu.mult, op1=Alu.add)
            lastd['dve'] = lb
            if t == t0 + nb - 1:
                # group complete: one SWDGE window DMA (diagonal-block AP)
                if t0 == 0:
                    wd = nc.gpsimd.dma_start(h[0:128, 0:132], tile_w[:, 4:WIN])
                elif t0 == NT - 1:
                    wd = nc.gpsimd.dma_start(h[r0:r0 + 128, r0 - 4:ND],
                                             tile_w[:, 0:132])
                else:
                    out_ap = bass.AP(
                        tensor=h, offset=128 * t0 * ND + 128 * t0 - 4,
                        ap=[[ND, 128], [128 * ND + 128, nb], [1, WIN]])
                    in_ap = tile_w[:].rearrange("p (b j) -> p b j", j=WIN)
                    wd = nc.gpsimd.dma_start(out_ap, in_ap)
                window_dmas.append(wd)

        # drive: chunk group g feeds blocks 4g..4g+3 (band regions for block
        # t live near EFT columns off_d + 128t, i.e. chunks {g, 4+g, 8+g,
        # 12+g} for g = t//4)
        # blocks lag one chunk group: a block's band slice can straddle into
        # the next chunk (RE cols off_d + 128t .. +131), which lands in
        # group g+1 for the d=1 region
        # block 0's lower band slices reach back into the previous band
        # region's tail (chunks 7 and 11, group 3), so it goes last
        for g in range(4):
            for j in (g, 4 + g, 8 + g, 12 + g):
                l1l2_edges(j)
            lastd['act'] = l1l2_nodes(g)
            if g >= 1:
                for t in range(4 * (g - 1), 4 * g):
                    if t != 0:
                        emit_block(t)
        for t in (12, 13, 14, 15, 0):
            emit_block(t)

        # ---- tail: let SP observe every active proc via single-wait nops so
        # the framework's kernel-end Drain has all its waits elided (the
        # CTRL struct holds few sync waits).
        tail = zero_dmas[-8:] + window_dmas
        tail += [lastd['pe'], lastd['act'], lastd['dve']]
        for dep in tail:
            n = nc.sync.nop(nofuse=True)
            add_dep_helper(n.ins, dep.ins, reason="tail drain wait split")

    return nc


def _get_program():
    global _PROG
    if _PROG is None:
        _PROG = _build_program()
    return _PROG


def _host_prep(inputs):
    nf = np.asarray(inputs["node_features"], dtype=np.float32)
    ef = np.asarray(inputs["edge_features"], dtype=np.float32)
    assert nf.shape == (B * NPG, HID), nf.shape
    assert ef.shape == (B * EP, HID), ef.shape

    wo1 = np.ascontiguousarray(np.asarray(inputs["Wo1"], np.float32))
    wc1 = np.ascontiguousarray(np.asarray(inputs["Wc1"], np.float32))
    bo1 = np.ascontiguousarray(np.asarray(inputs["bo1"], np.float32).reshape(HID, 1))
    bc1 = np.ascontiguousarray(np.asarray(inputs["bc1"], np.float32).reshape(HID, 1))
    wo2 = np.asarray(inputs["Wo2"], np.float32).reshape(HID)
    wc2 = np.asarray(inputs["Wc2"], np.float32).reshape(HID)
    bo2 = float(np.asarray(inputs["bo2"]).reshape(()))
    bc2 = float(np.asarray(inputs["bc2"]).reshape(()))
    w2 = np.ascontiguousarray(np.stack([wc2, wo2], axis=1))  # [128, 2]

    # mask0[p, j'] = 1 iff j' == p + 8 ; band-g mask is mask0[:, 8-g : 8-g+136]
    p = np.arange(128)[:, None]
    jp = np.arange(144)[None, :]
    mask0 = (jp == p + 8).astype(np.float32)

    # biasc[p, 9t+g]: +bo2+1e-6 on the diagonal band (g=4), +bc2 on couplings
    row9 = np.array([bc2] * 4 + [bo2 + 1e-6] + [bc2] * 4, np.float32)
    biasc = np.broadcast_to(np.tile(row9, NT), (128, 9 * NT))
    biasc = np.ascontiguousarray(biasc)

    shared = dict(wo1=wo1, wc1=wc1, bo1=bo1, bc1=bc1, w2=w2,
                  mask0=mask0, biasc=biasc,
                  zv=np.zeros((128, 1916), np.float32))

    in_maps = []
    for b in range(B):
        x_b = nf[b * NPG + 2:(b + 1) * NPG]                    # [2048, 128]
        ef_b = ef[b * EP:(b + 1) * EP]                         # [8182, 128]
        eft = np.zeros((HID, EPAD), np.float32)
        eft[:, :EP] = ef_b.T
        m = dict(shared)
        m["xt"] = np.ascontiguousarray(x_b.T)
        m["eft"] = eft
        in_maps.append(m)
    return in_maps


def kernel(**inputs):
    import sys
    if "/opt/trn_rl_repo" not in sys.path:
        sys.path.insert(0, "/opt/trn_rl_repo")
    from concourse.bass_utils import run_bass_kernel_spmd

    nc = _get_program()
    in_maps = _host_prep(inputs)
    res = run_bass_kernel_spmd(nc, in_maps, core_ids=list(range(B)))
    out = np.stack([np.asarray(res.results[i]["h"]) for i in range(B)], axis=0)
    return out.astype(np.float32)



# revision 12
# speedup vs baseline: 139.3010x; 139.3010x over previous
"""DNA Transport Hamiltonian GNN kernel for Trainium2 (8 NeuronCores).

Builds [8, 2048, 2048] banded Hamiltonians (9 diagonals; 99.6% zeros).
Sharding: one graph per core; MLP weights replicated.

v2 design: the framework pre-zeroes & donates ExternalOutput buffers
(see run_bass_via_pjrt: "kernels that don't write every element rely on
that"), so the kernel writes ONLY the 9-diagonal band (~74KB/core) via
diagonal-stride DMA APs instead of streaming the 16MB zero background.
Features/weights are fp16 (tolerance 2e-2; fp16 adds ~1e-3), halving
input DMA and running all matmuls in 1-pass mode.

Layer-2 is computed directly in partition-major order: for each 128-row
block t and diagonal g, a "stationary-window" matmul
  c[p, g] = sum_hid H1[hid, w_g + p] * w2[hid]
with stationary = a 128-col window of the relu'd layer-1 activations.
This eliminates the row-major layer-2 + 144 PE transposes + 128 masked
DVE window-assembly ops of the previous version.

Hardcoded problem structure (from the generating module):
  B=8 graphs, 2048 DNA nodes/graph (+2 contact nodes at graph start),
  HID=128, edges per graph: (i, i+d) for d=1..4, d-major layout,
  8182 edges/graph, graphs contiguous.
"""

import numpy as np

B = 8
ND = 2048            # DNA nodes per graph == H_size
NPG = ND + 2         # nodes per graph incl. 2 contacts
HID = 128
EP = 8182            # edges per graph
EW = 8192            # EFT width: 4-col head pad + edges + tail pad
NT = ND // 128       # 16 row blocks
OFF = {1: 0, 2: 2047, 3: 4093, 4: 6138}   # start of band d in edge order

_PROG = None


def _block_deps(t):
    """EFT 512-chunks and XT 512-chunk needed by block t's windows."""
    r0 = 128 * t
    cs = set()
    for d in range(1, 5):
        lo = 4 + OFF[d] + r0 - d          # lower-diag window start
        hi = 4 + OFF[d] + r0 + 127        # upper-diag window end
        for c in range(lo // 512, hi // 512 + 1):
            cs.add(c)
    return cs, t // 4


def _build_program():
    import concourse.bass as bass
    import concourse.tile as tile
    from concourse.tile import add_dep_helper
    from concourse import mybir
    from contextlib import ExitStack

    f32 = mybir.dt.float32
    f16 = mybir.dt.float16
    Alu = mybir.AluOpType
    Act = mybir.ActivationFunctionType

    nc = bass.Bass()

    eft = nc.declare_dram_parameter("eft", [HID, EW], f16, isOutput=False)
    xt = nc.declare_dram_parameter("xt", [HID, ND], f16, isOutput=False)
    ws = nc.declare_dram_parameter("ws", [HID, 258], f16, isOutput=False)
    bs = nc.declare_dram_parameter("bs", [HID, 182], f32, isOutput=False)
    h = nc.declare_dram_parameter("h", [ND, ND], f32, isOutput=True)

    with tile.TileContext(nc) as tc, ExitStack() as ctx:
        cons = ctx.enter_context(tc.tile_pool(name="cons", bufs=1))
        psL1 = ctx.enter_context(tc.tile_pool(name="psL1", bufs=3, space="PSUM"))
        psPers = ctx.enter_context(tc.tile_pool(name="psPers", bufs=1, space="PSUM"))

        EFT = cons.tile([HID, EW], f16)
        XT = cons.tile([HID, ND], f16)
        WS = cons.tile([HID, 258], f16)
        BS = cons.tile([HID, 182], f32)
        H1ET = cons.tile([HID, EW], f16)
        H1XT = cons.tile([HID, ND], f16)
        CW = cons.tile([128, 126], f32)      # bias-added c tiles, blocks 1..14
        CWE = cons.tile([128, 18], f32)      # bias-added c tiles, blocks 15, 0
        SCRA = cons.tile([1, 2], f32)        # ACT warmup scratch
        SCRD = cons.tile([1, 2], f32)        # DVE warmup scratch

        # ---- input DMAs (HWDGE, queues round-robin by issue order).
        # Wave 0 chunks (J=0,2,4,6 + XT J0) land first so layer-1 can start
        # while wave 1 (J=1,3,5,7 + XT J1) is still in flight.
        hw = []
        hw.append(nc.sync.dma_start(WS[:], ws[:]))
        hw.append(nc.sync.dma_start(BS[:], bs[:]))
        EJ = {}
        XJ = {}
        for J in (0, 2, 4, 6):
            EJ[J] = nc.sync.dma_start(EFT[:, 1024 * J:1024 * (J + 1)],
                                      eft[:, 1024 * J:1024 * (J + 1)])
            hw.append(EJ[J])
        XJ[0] = nc.sync.dma_start(XT[:, 0:1024], xt[:, 0:1024])
        hw.append(XJ[0])
        for J in (1, 3, 5, 7):
            EJ[J] = nc.sync.dma_start(EFT[:, 1024 * J:1024 * (J + 1)],
                                      eft[:, 1024 * J:1024 * (J + 1)])
            hw.append(EJ[J])
        XJ[1] = nc.sync.dma_start(XT[:, 1024:2048], xt[:, 1024:2048])
        hw.append(XJ[1])

        # ---- engine warmups (absorb DMA-queue semaphores with single-wait
        # ops so later instructions — esp. PE matmuls and DMAs, which take
        # one sync wait — never need >1).
        nc.scalar.activation(SCRA[0:1, 0:1], BS[0:1, 0:1], Act.Copy,
                             bias=0.0, scale=0.0)
        nc.vector.tensor_copy(SCRD[0:1, 0:1], BS[0:1, 0:1])

        pd = psPers.tile([1, 16], f32)
        # Persistent c-tile banks with no column reuse (PE 1-sync-wait
        # limit: no WAR waits on window matmuls). The tile framework
        # treats PSUM reads as RMW at tile granularity, so each bank gets
        # exactly ONE fused DVE bias-add reading it: PSC (blocks 1..14,
        # read after block 14 so the mid-band DMA overlaps blocks 15/0)
        # and PSCE (blocks 15 and 0, read at the end).
        PSC = psPers.tile([128, 140], f32)   # 14 blocks x 9 + 14 dummy cols
        PSCE = psPers.tile([128, 20], f32)   # blocks 15, 0 + 2 dummy cols
        wcol = [0]

        def warm(tile_, col):
            nc.tensor.matmul(pd[0:1, wcol[0]:wcol[0] + 1],
                             tile_[0:1, col:col + 1], tile_[0:1, col:col + 1],
                             start=True, stop=True)
            wcol[0] += 1

        warm(WS, 0)

        lastd = {}
        dve_order = []   # EFT chunks relu'd on DVE, in program order

        def l1_eft(c):
            ps = psL1.tile([128, 512], f32)
            nc.tensor.matmul(ps[:], WS[:, 0:128], EFT[:, 512 * c:512 * (c + 1)],
                             start=True, stop=True)
            if c % 2 == 0:
                lastd['act'] = nc.scalar.activation(
                    H1ET[:, 512 * c:512 * (c + 1)], ps[:], Act.Relu,
                    bias=BS[:, 0:1])
            else:
                dve_order.append(c)
                lastd['dve'] = nc.vector.tensor_scalar(
                    H1ET[:, 512 * c:512 * (c + 1)], ps[:], BS[:, 0:1], 0.0,
                    op0=Alu.add, op1=Alu.max)

        def l1_xt(g):
            ps = psL1.tile([128, 512], f32)
            nc.tensor.matmul(ps[:], WS[:, 128:256], XT[:, 512 * g:512 * (g + 1)],
                             start=True, stop=True)
            lastd['act'] = nc.scalar.activation(
                H1XT[:, 512 * g:512 * (g + 1)], ps[:], Act.Relu,
                bias=BS[:, 1:2])

        # window-matmul emission order: g=4 (XT, latest ACT chunk) first
        GORD = (4, 5, 3, 6, 2, 7, 1, 8, 0)

        def emit_block(t):
            r0 = 128 * t
            if 1 <= t <= 14:
                pst, c0, dcol = PSC, 9 * (t - 1), 126 + (t - 1)
            else:
                pst, c0, dcol = PSCE, (0 if t == 15 else 9), 18 + (t % 2)
            # dummy matmul: absorbs the DVE semaphore (this block's
            # DVE-relu'd chunks) so the real window matmuls wait only on ACT
            deps, _ = _block_deps(t)
            dcs = [c for c in dve_order if c in deps]
            if dcs:
                dc = 512 * dcs[-1] + 1
                nc.tensor.matmul(pst[0:1, dcol:dcol + 1], H1ET[0:1, dc:dc + 1],
                                 H1ET[0:1, dc:dc + 1], start=True, stop=True)
            else:
                nc.tensor.matmul(pst[0:1, dcol:dcol + 1], SCRD[0:1, 0:1],
                                 SCRD[0:1, 0:1], start=True, stop=True)
            for g in GORD:
                if g == 4:
                    lhsT = H1XT[:, r0:r0 + 128]
                    mov = WS[:, 257:258]
                else:
                    d = g - 4 if g > 4 else 4 - g
                    w0 = 4 + OFF[d] + r0 - (d if g < 4 else 0)
                    lhsT = H1ET[:, w0:w0 + 128]
                    mov = WS[:, 256:257]
                lastd['pe'] = nc.tensor.matmul(pst[:, c0 + g:c0 + g + 1],
                                               lhsT, mov,
                                               start=True, stop=True)

        # ---- schedule: wave 0 -> blocks 1..7, wave 1 -> 8..15 then 0
        done_e, done_x = set(), set()
        emitted = set()
        WAVES = [((0, 2, 4, 6), (0,)), ((1, 3, 5, 7), (1,))]
        out_dmas = []

        def ready_blocks():
            out = []
            for t in list(range(1, NT)) + [0]:
                if t in emitted:
                    continue
                cs, xg = _block_deps(t)
                if cs <= done_e and xg in done_x:
                    out.append(t)
            return out

        for eJs, xJs in WAVES:
            for J in eJs:
                warm(EFT, 1024 * J)
                l1_eft(2 * J)
                l1_eft(2 * J + 1)
            for J in xJs:
                warm(XT, 1024 * J)
                l1_xt(2 * J)
                l1_xt(2 * J + 1)
                done_x.update((2 * J, 2 * J + 1))
            for J in eJs:
                done_e.update((2 * J, 2 * J + 1))
            for t in ready_blocks():
                emit_block(t)
                emitted.add(t)
                if t == 14:
                    # blocks 1..14 done: ONE fused bias-add over their PSUM
                    # bank, then one diagonal-AP DMA covering rows 128..1919
                    # (overlaps blocks 15/0 compute)
                    lastd['dve'] = nc.vector.tensor_tensor(
                        CW[:], PSC[:, 0:126], BS[:, 2:128], op=Alu.add)
                    out_ap = bass.AP(
                        tensor=h, offset=128 * (ND + 1) - 4,
                        ap=[[ND + 1, 128], [128 * (ND + 1), 14], [1, 9]])
                    in_ap = CW[:].rearrange("p (b g) -> p b g", g=9)
                    out_dmas.append(nc.gpsimd.dma_start(out_ap, in_ap))

        assert emitted == set(range(NT)), emitted

        # ---- edge blocks 15 (CWE cols 0:9) and 0 (cols 9:18): mask the
        # out-of-band entries to EXACT zeros, then add the (masked) bias.
        # Corner rows can then be written with full 9-wide diagonal
        # windows whose spill cells are exact zeros landing in zero
        # regions of h — no per-segment corner DMAs needed.
        nc.vector.tensor_tensor(CWE[:], PSCE[:, 0:18], BS[:, 146:164],
                                op=Alu.mult)
        lastd['dve'] = nc.vector.tensor_tensor(
            CWE[:], CWE[:], BS[:, 164:182], op=Alu.add)
        # rows 1920..2043: full 9-wide diagonal windows
        out_dmas.append(nc.gpsimd.dma_start(
            bass.AP(tensor=h, offset=1920 * (ND + 1) - 4,
                    ap=[[ND + 1, 124], [1, 9]]),
            CWE[0:124, 0:9]))
        # rows 2044..2046: 9-wide; masked-zero tail spills into the next
        # row's zero region (in bounds)
        out_dmas.append(nc.gpsimd.dma_start(
            bass.AP(tensor=h, offset=2044 * (ND + 1) - 4,
                    ap=[[ND + 1, 3], [1, 9]]),
            CWE[124:127, 0:9]))
        # row 2047: 5-wide ends exactly at h's last element
        out_dmas.append(nc.gpsimd.dma_start(
            bass.AP(tensor=h, offset=2047 * (ND + 1) - 4,
                    ap=[[ND + 1, 1], [1, 5]]),
            CWE[127:128, 0:5]))
        # rows 4..127: full 9-wide diagonal windows
        out_dmas.append(nc.gpsimd.dma_start(
            bass.AP(tensor=h, offset=4 * ND,
                    ap=[[ND + 1, 124], [1, 9]]),
            CWE[4:128, 9:18]))
        # rows 1..3: 9-wide; masked-zero head spills into the previous
        # row's zero tail
        out_dmas.append(nc.gpsimd.dma_start(
            bass.AP(tensor=h, offset=1 * (ND + 1) - 4,
                    ap=[[ND + 1, 3], [1, 9]]),
            CWE[1:4, 9:18]))
        # row 0: diag+upper 5-wide at cols 0..4
        out_dmas.append(nc.gpsimd.dma_start(
            bass.AP(tensor=h, offset=0,
                    ap=[[ND + 1, 1], [1, 5]]),
            CWE[0:1, 13:18]))

        # ---- tail: SP observes every outstanding proc via single-wait nops
        # so the framework's kernel-end Drain has its waits elided.
        tail = hw[-8:] + out_dmas + [lastd['pe'], lastd['act'], lastd['dve']]
        for dep in tail:
            n = nc.sync.nop(nofuse=True)
            add_dep_helper(n.ins, dep.ins, reason="tail drain wait split")

    return nc


def _get_program():
    global _PROG
    if _PROG is None:
        _PROG = _build_program()
    return _PROG


def _host_prep(inputs):
    nf = np.asarray(inputs["node_features"], dtype=np.float32)
    ef = np.asarray(inputs["edge_features"], dtype=np.float32)
    assert nf.shape == (B * NPG, HID), nf.shape
    assert ef.shape == (B * EP, HID), ef.shape

    wo1 = np.asarray(inputs["Wo1"], np.float32)
    wc1 = np.asarray(inputs["Wc1"], np.float32)
    bo1 = np.asarray(inputs["bo1"], np.float32).reshape(HID)
    bc1 = np.asarray(inputs["bc1"], np.float32).reshape(HID)
    wo2 = np.asarray(inputs["Wo2"], np.float32).reshape(HID)
    wc2 = np.asarray(inputs["Wc2"], np.float32).reshape(HID)
    bo2 = float(np.asarray(inputs["bo2"]).reshape(()))
    bc2 = float(np.asarray(inputs["bc2"]).reshape(()))

    ws = np.concatenate(
        [wc1, wo1, wc2[:, None], wo2[:, None]], axis=1).astype(np.float16)
    ws = np.ascontiguousarray(ws)                       # [128, 258]
    row9 = np.array([bc2] * 4 + [bo2 + 1e-6] + [bc2] * 4, np.float32)
    bs = np.empty((HID, 182), np.float32)
    bs[:, 0] = bc1
    bs[:, 1] = bo1
    bs[:, 2:146] = np.tile(row9, 16)[None, :]
    # edge-block validity mask [128, 18]: cols 0:9 block 15, 9:18 block 0
    maske = np.ones((HID, 18), np.float32)
    for k in range(4):
        p = 124 + k                  # block-15 row r = 2044+k
        maske[p, 8 - k:9] = 0.0      # upper diags beyond col 2047
        maske[k, 9:9 + 4 - k] = 0.0  # block-0 row k: lower diags r < d
    bs[:, 146:164] = maske
    bs[:, 164:182] = np.tile(row9, 2)[None, :] * maske
    bs = np.ascontiguousarray(bs)

    shared = dict(ws=ws, bs=bs)
    in_maps = []
    for b in range(B):
        x_b = nf[b * NPG + 2:(b + 1) * NPG]             # [2048, 128]
        ef_b = ef[b * EP:(b + 1) * EP]                  # [8182, 128]
        eft = np.zeros((HID, EW), np.float16)
        eft[:, 4:4 + EP] = ef_b.T.astype(np.float16)
        m = dict(shared)
        m["eft"] = eft
        m["xt"] = np.ascontiguousarray(x_b.T.astype(np.float16))
        in_maps.append(m)
    return in_maps


def kernel(**inputs):
    import sys
    if "/opt/trn_rl_repo" not in sys.path:
        sys.path.insert(0, "/opt/trn_rl_repo")
    from concourse.bass_utils import run_bass_kernel_spmd

    nc = _get_program()
    in_maps = _host_prep(inputs)
    res = run_bass_kernel_spmd(nc, in_maps, core_ids=list(range(B)))
    out = np.stack([np.asarray(res.results[i]["h"]) for i in range(B)], axis=0)
    return out.astype(np.float32)
